# revision 2
# baseline (speedup 1.0000x reference)
"""Trainium2 Bass kernel for nn_Attention_36137854828870 (v2).

Multi-head causal attention with rotary embeddings, 8 cores:
data-parallel over batch (4) x tensor-parallel over head groups (2x8 heads).
Core c: batch c//2, head group c%2; host sums the two partial outputs.

v2 vs baseline: all-bf16 dataflow (fp32 PSUM), Q/K fully SBUF-resident
(permA rope + stride-2-partition SBUF->SBUF DMA shuffle to head-contiguous
permB; no DRAM staging), chunk-interleaved schedule (proj chunk c ->
wo(qb c-1) -> attention qb c at QW=512), paired exp instructions, causal
mask via gpsimd.affine_select, evictions on Pool/DVE so ScalarE only
runs exp.
"""

import os
import sys

sys.path.insert(0, "/opt/trn_rl_repo")

import numpy as np
import ml_dtypes

import concourse.bass as bass
import concourse.mybir as mybir
import concourse.tile as tile
from concourse import bacc
from concourse.bass_utils import run_bass_kernel_spmd

B, S, D = 4, 2048, 1024
H, HD = 16, 64
P = 128
NCORES = 8
HPC = H // 2          # 8 heads per core
DG = HPC * HD         # 512
NKT = D // P          # 8 contraction tiles for projections
NDT = DG // P         # 4 partition-tiles of Q/K/attnT
NSC = 4               # s-chunks == q-blocks
CW = S // NSC         # 512 chunk/q-block width
NST = S // P          # 16 key tiles
F32 = mybir.dt.float32
BF16 = mybir.dt.bfloat16

_PROGRAM = None


def _build_program():
    nc = bacc.Bacc("TRN2", target_bir_lowering=False, debug=False)

    xT_d = nc.dram_tensor("xT", [D, S], BF16, kind="ExternalInput")
    wq_d = nc.dram_tensor("wq", [NDT, P, NKT, P], BF16, kind="ExternalInput")
    wk_d = nc.dram_tensor("wk", [NDT, P, NKT, P], BF16, kind="ExternalInput")
    wv_d = nc.dram_tensor("wv", [D, DG], BF16, kind="ExternalInput")
    wo_d = nc.dram_tensor("wo", [DG, D], BF16, kind="ExternalInput")
    cos_d = nc.dram_tensor("cost", [P, S], BF16, kind="ExternalInput")
    sin_d = nc.dram_tensor("sint", [P, S], BF16, kind="ExternalInput")
    y_d = nc.dram_tensor("y", [S, D], BF16, kind="ExternalOutput")
    DBG = os.environ.get("KV2_DEBUG") == "1"
    if DBG:
        dbg_ktb = nc.dram_tensor("dbg_ktb", [P, NDT, S], BF16, kind="ExternalOutput")
        dbg_qtb = nc.dram_tensor("dbg_qtb", [P, NDT, S], BF16, kind="ExternalOutput")
        dbg_v = nc.dram_tensor("dbg_v", [P, NST, HPC, HD + 1], BF16, kind="ExternalOutput")
        dbg_attnT = nc.dram_tensor("dbg_attnT", [P, NDT, S], BF16, kind="ExternalOutput")
        dbg_ltile = nc.dram_tensor("dbg_ltile", [P, 4, S], BF16, kind="ExternalOutput")

    xT_v = xT_d.ap().rearrange("(kt p) s -> p kt s", p=P)
    wq_v = wq_d.ap().rearrange("dt p kt m -> p dt kt m")
    wk_v = wk_d.ap().rearrange("dt p kt m -> p dt kt m")
    wv_v = wv_d.ap().rearrange("(kt p) m -> p kt m", p=P)
    wo_v = wo_d.ap().rearrange("(dt p) n -> p dt n", p=P)

    with tile.TileContext(nc) as tc:
        with tc.tile_pool(name="big", bufs=1) as big, \
             tc.tile_pool(name="wres", bufs=1) as wres, \
             tc.tile_pool(name="trig", bufs=1) as trig:
            # persistent SBUF tensors
            V = big.tile([P, NST, HPC, HD + 1], BF16, tag="V")
            KTb = big.tile([P, NDT, S], BF16, tag="KTb")
            QTb = big.tile([P, NDT, S], BF16, tag="QTb")
            attnT = big.tile([P, NDT, S], BF16, tag="attnT")
            ltile = big.tile([P, 4, S], BF16, tag="ltile")
            ones = big.tile([P, NST * HPC], BF16, tag="ones")
            onecol = big.tile([P, HD], BF16, tag="onecol")
            nc.any.memset(ones[:], 1.0)
            nc.any.memset(onecol[:], 1.0)
            nc.vector.tensor_copy(
                V[:, :, :, HD : HD + 1],
                ones[:].rearrange("p (a b) -> p a b", a=NST),
            )

            wqt = wres.tile([P, NDT, NKT, P], BF16, tag="wq")
            wkt = wres.tile([P, NDT, NKT, P], BF16, tag="wk")
            wvt = wres.tile([P, NKT, DG], BF16, tag="wv")
            wo_sb = wres.tile([P, NDT, D], BF16, tag="wo")
            cost = trig.tile([P, S], BF16, tag="cos")
            sint = trig.tile([P, S], BF16, tag="sin")

            with tc.tile_pool(name="xw", bufs=2) as xw, \
                 tc.tile_pool(name="prj", bufs=2) as prj, \
                 tc.tile_pool(name="scr", bufs=2) as scr, \
                 tc.tile_pool(name="expool", bufs=6) as expool, \
                 tc.tile_pool(name="npool", bufs=2) as npool, \
                 tc.tile_pool(name="ypool", bufs=3) as ypool, \
                 tc.tile_pool(name="ps1", bufs=2, space="PSUM") as ps1, \
                 tc.tile_pool(name="apsum", bufs=2, space="PSUM") as apsum, \
                 tc.tile_pool(name="opsum", bufs=2, space="PSUM") as opsum:

                # first-chunk critical loads: the first proj_dt(dt=0)
                # needs wkt col-slice 0 + xc0 kt-by-kt; split finely and
                # spread across queues, first-used first
                xc0 = xw.tile([P, NKT, CW], BF16, tag="xc")
                nc.scalar.dma_start(out=wkt[:, 0, 0:4, :], in_=wk_v[:, 0, 0:4, :])
                nc.sync.dma_start(out=wkt[:, 0, 4:8, :], in_=wk_v[:, 0, 4:8, :])
                for kt in range(0, NKT, 2):
                    q = (nc.sync, nc.scalar)[(kt // 2) % 2]
                    q.dma_start(
                        out=xc0[:, kt : kt + 2, :], in_=xT_v[:, kt : kt + 2, 0:CW]
                    )
                nc.sync.dma_start(out=cost[:, 0:CW], in_=cos_d.ap()[:, 0:CW])
                nc.scalar.dma_start(out=sint[:, 0:CW], in_=sin_d.ap()[:, 0:CW])
                for dt in range(1, NDT):
                    (nc.sync, nc.scalar)[dt % 2].dma_start(
                        out=wkt[:, dt], in_=wk_v[:, dt]
                    )
                for dt in range(NDT):
                    (nc.sync, nc.scalar)[dt % 2].dma_start(
                        out=wqt[:, dt], in_=wq_v[:, dt]
                    )
                nc.sync.dma_start(out=wvt[:], in_=wv_v[:])

                def proj_dt(wt, xc, ta, dt):
                    psq = ps1.tile([P, CW], F32, tag="ps")
                    for kt in range(NKT):
                        nc.tensor.matmul(
                            psq[:],
                            wt[:, dt, kt, :],
                            xc[:, kt, :],
                            start=(kt == 0),
                            stop=(kt == NKT - 1),
                        )
                    nc.vector.tensor_copy(ta[:, dt, :], psq[:])

                def rope(ta, csl):
                    # permA pairing: tiles (dt, dt+2) hold even/odd head-dims
                    # of the same heads, lane-aligned
                    for dt in range(2):
                        a0 = ta[:, dt, :]
                        a1 = ta[:, dt + 2, :]
                        cc = cost[:, csl]
                        ss = sint[:, csl]
                        tt = scr.tile([P, CW], BF16, tag="t")
                        uu = scr.tile([P, CW], BF16, tag="u")
                        nc.vector.tensor_mul(tt[:], a0, ss)
                        nc.vector.tensor_mul(uu[:], a1, cc)
                        nc.vector.tensor_mul(a0, a0, cc)
                        nc.vector.tensor_mul(a1, a1, ss)
                        nc.vector.tensor_sub(a0, a0, a1)
                        nc.vector.tensor_add(a1, tt[:], uu[:])

                def shuffle(ta, dstb, csl, qnum):
                    """permA -> permB (head-contiguous) via stride-2-partition
                    SBUF->SBUF DMA.  permB tile dtb rows: head 2dtb at 0-63,
                    head 2dtb+1 at 64-127, even dims on even rows.  Source:
                    permA tile dtb//2 (+2 for odds) rows [64*(dtb%2), +64)."""
                    for dtb in range(NDT):
                        rlo = 64 * (dtb % 2)
                        for par in range(2):
                            nc.sync.dma_start(
                                out=dstb[par : P : 2, dtb, csl],
                                in_=ta[rlo : rlo + 64, dtb // 2 + 2 * par, :],
                            )

                CY = 1.0 / 2.4
                sched = {"pe": 0.0, "act": 0.0}
                pend = []  # exp-done times of in-flight pss tiles (max 2)

                def attention(c, units, post_head=None):
                    """Attention for q-block c.  `units` are (pe_cost, thunk)
                    pairs of independent work (proj/wo for other blocks).  A
                    build-time cursor model of PE and ScalarE completion times
                    pops units exactly where the in-order PE stream would
                    otherwise stall on exp results or PSUM-buffer recycling."""
                    kr = (c + 1) * CW
                    njt = kr // P
                    qb0 = c * CW
                    qsl = slice(qb0, qb0 + CW)
                    units = list(units)[::-1]
                    npairs = (NDT * 2 * njt) // 2
                    frac = {"acc": 0.0, "per": len(units) / max(npairs, 1)}

                    def need(t):
                        while units and sched["pe"] < t:
                            ucost, u = units.pop()
                            u()
                            sched["pe"] += ucost

                    for dtb in range(NDT):
                        for hh in range(2):
                            pb = hh * 64
                            h = dtb * 2 + hh
                            pso = opsum.tile([P, CW], F32, tag="pso")

                            def emit_pv(j, qlo, ex, slot):
                                nc.tensor.matmul(
                                    pso[0 : HD + 1, qlo:CW],
                                    V[:, j, h, :],
                                    ex[:, slot, qlo:CW],
                                    start=(j == 0),
                                    stop=(j == njt - 1),
                                )

                            pipe = []
                            for jp in range(njt // 2):
                                j0, j1 = 2 * jp, 2 * jp + 1
                                d0 = j0 >= njt - 4
                                d1 = j1 >= njt - 4
                                qlo0 = (j0 - (njt - 4)) * P if d0 else 0
                                qlo1 = (j1 - (njt - 4)) * P if d1 else 0
                                # pss pool bufs=2: this tile recycles the one
                                # whose exp finished 2 pairs ago
                                if len(pend) >= 2:
                                    need(pend.pop(0))
                                pss = apsum.tile([P, 2, CW], F32, tag="pss")
                                nc.tensor.matmul(
                                    pss[:, 0, qlo0:CW],
                                    KTb[pb : pb + 64, dtb, j0 * P : (j0 + 1) * P],
                                    QTb[pb : pb + 64, dtb, qb0 + qlo0 : qb0 + CW],
                                    start=True,
                                    stop=True,
                                )
                                nc.tensor.matmul(
                                    pss[:, 1, qlo1:CW],
                                    KTb[pb : pb + 64, dtb, j1 * P : (j1 + 1) * P],
                                    QTb[pb : pb + 64, dtb, qb0 + qlo1 : qb0 + CW],
                                    start=True,
                                    stop=True,
                                )
                                sched["pe"] += (2 * CW - qlo0 - qlo1) * CY
                                ex = expool.tile([P, 2, CW], BF16, tag="ex")
                                # one exp covers the pair; for a diag pair the
                                # j1 slice [qlo0, qlo1) reads stale PSUM --
                                # bounded garbage, zeroed by the memset below
                                nc.scalar.activation(
                                    ex[:, :, qlo0:CW],
                                    pss[:, :, qlo0:CW],
                                    mybir.ActivationFunctionType.Exp,
                                    scale=float(1.0 / np.sqrt(HD)),
                                )
                                edone = max(sched["act"], sched["pe"]) + (
                                    2 * (CW - qlo0) + 222
                                ) * 0.833 + 180
                                sched["act"] = edone
                                pend.append(edone)
                                if d1:
                                    # one select covers the stale-garbage
                                    # block [qlo0,qlo1) (i-p-128<0 there) and
                                    # the diagonal triangle of j1
                                    w = qlo1 + P - qlo0
                                    nc.gpsimd.affine_select(
                                        out=ex[:, 1, qlo0 : qlo1 + P],
                                        in_=ex[:, 1, qlo0 : qlo1 + P],
                                        compare_op=mybir.AluOpType.is_ge,
                                        fill=0.0,
                                        base=qlo0 - qlo1,
                                        pattern=[[1, w]],
                                        channel_multiplier=-1,
                                    )
                                if d0:
                                    nc.gpsimd.affine_select(
                                        out=ex[:, 0, qlo0 : qlo0 + P],
                                        in_=ex[:, 0, qlo0 : qlo0 + P],
                                        compare_op=mybir.AluOpType.is_ge,
                                        fill=0.0,
                                        base=0,
                                        pattern=[[1, P]],
                                        channel_multiplier=-1,
                                    )
                                eready = edone + (900 if (d0 or d1) else 0)
                                pipe.append((eready, (j0, qlo0, ex, 0), (j1, qlo1, ex, 1)))
                                if len(pipe) > 2:
                                    eready, a, b = pipe.pop(0)
                                    need(eready)
                                    emit_pv(*a)
                                    emit_pv(*b)
                                    sched["pe"] += (2 * CW - a[1] - b[1]) * CY
                            for eready, a, b in pipe:
                                need(eready)
                                emit_pv(*a)
                                emit_pv(*b)
                                sched["pe"] += (2 * CW - a[1] - b[1]) * CY

                            with nc.allow_low_precision(
                                reason="1/l applied to bf16 attnT; bf16 is enough"
                            ):
                                nc.vector.reciprocal(
                                    ltile[
                                        (h % 2) * 64 : (h % 2) * 64 + 1, h // 2, qsl
                                    ],
                                    pso[HD : HD + 1, :],
                                )
                            nc.vector.tensor_copy(
                                attnT[pb : pb + HD, dtb, qsl], pso[0:HD, :]
                            )
                            if post_head is not None:
                                post_head(dtb, hh)

                    while units:
                        ucost, u = units.pop()
                        u()
                        sched["pe"] += ucost

                def normalize_head_fast(c, dtb, hh):
                    qsl = slice(c * CW, (c + 1) * CW)
                    h = dtb * 2 + hh
                    pb = hh * 64
                    bcp = opsum.tile([P, CW], F32, tag="pso")
                    nc.tensor.matmul(
                        bcp[0:HD, :],
                        onecol[(h % 2) * 64 : (h % 2) * 64 + 1, :],
                        ltile[(h % 2) * 64 : (h % 2) * 64 + 1, h // 2, qsl],
                        start=True,
                        stop=True,
                    )
                    nc.vector.tensor_mul(
                        attnT[pb : pb + HD, dtb, qsl],
                        attnT[pb : pb + HD, dtb, qsl],
                        bcp[0:HD, :],
                    )

                def normalize_head(c, dtb, hh):
                    qsl = slice(c * CW, (c + 1) * CW)
                    h = dtb * 2 + hh
                    pb = hh * 64
                    bc = npool.tile([P, CW], BF16, tag="bc")
                    nc.sync.dma_start(
                        out=bc[pb : pb + HD, :],
                        in_=ltile[(h % 2) * 64 : (h % 2) * 64 + 1, h // 2, qsl]
                        .unsqueeze(1)
                        .broadcast_to((1, HD, CW)),
                    )
                    nc.vector.tensor_mul(
                        attnT[pb : pb + HD, dtb, qsl],
                        attnT[pb : pb + HD, dtb, qsl],
                        bc[pb : pb + HD, :],
                    )

                def normalize_dtb(c, dtb):
                    qsl = slice(c * CW, (c + 1) * CW)
                    bc = npool.tile([P, CW], BF16, tag="bc")
                    for hh in range(2):
                        h = dtb * 2 + hh
                        nc.sync.dma_start(
                            out=bc[hh * 64 : hh * 64 + HD, :],
                            in_=ltile[
                                (h % 2) * 64 : (h % 2) * 64 + 1, h // 2, qsl
                            ]
                            .unsqueeze(1)
                            .broadcast_to((1, HD, CW)),
                        )
                    nc.vector.tensor_mul(
                        attnT[:, dtb, qsl], attnT[:, dtb, qsl], bc[:]
                    )

                def normalize(c):
                    for dtb in range(NDT):
                        normalize_dtb(c, dtb)

                def wo_qt(c, qt, nt):
                    qlo = c * CW + qt * P
                    psy = ps1.tile([P, CW], F32, tag="ps")
                    for dt in range(NDT):
                        nc.tensor.matmul(
                            psy[:],
                            attnT[:, dt, qlo : qlo + P],
                            wo_sb[:, dt, nt * CW : (nt + 1) * CW],
                            start=(dt == 0),
                            stop=(dt == NDT - 1),
                        )
                    yt = ypool.tile([P, CW], BF16, tag="yt")
                    nc.vector.tensor_copy(yt[:], psy[:])
                    nc.sync.dma_start(
                        out=y_d.ap()[qlo : qlo + P, nt * CW : (nt + 1) * CW],
                        in_=yt[:],
                    )

                def proj_v(xc, c, st):
                    psv = ps1.tile([P, CW], F32, tag="ps")
                    for kt in range(NKT):
                        nc.tensor.matmul(
                            psv[:],
                            xc[:, kt, st * P : (st + 1) * P],
                            wvt[:, kt, :],
                            start=(kt == 0),
                            stop=(kt == NKT - 1),
                        )
                    nc.vector.tensor_copy(
                        V[:, c * 4 + st, :, 0:HD],
                        psv[:].rearrange("p (h d) -> p h d", h=HPC),
                    )

                def proj_units(xc, c):
                    """Thunks projecting chunk c (K+rope+shuffle, Q likewise,
                    V) from an already-loading xc tile."""
                    csl = slice(c * CW, (c + 1) * CW)
                    units = []
                    for wt, dstb, qnum in ((wkt, KTb, 0), (wqt, QTb, 1)):
                        ta = prj.tile([P, NDT, CW], BF16, tag="ta")
                        for dt in range(NDT):
                            units.append(
                                (1707, lambda wt=wt, xc=xc, ta=ta, dt=dt: proj_dt(
                                    wt, xc, ta, dt
                                ))
                            )
                        units.append((0, lambda ta=ta, csl=csl: rope(ta, csl)))
                        units.append(
                            (0, lambda ta=ta, dstb=dstb, csl=csl, qnum=qnum: shuffle(
                                ta, dstb, csl, qnum
                            ))
                        )
                    for st in range(4):
                        units.append(
                            (1707, lambda xc=xc, c=c, st=st: proj_v(xc, c, st))
                        )
                    return units

                def wo_units(c):
                    units = []
                    for qt in range(4):
                        for nt in range(2):
                            units.append(
                                (853, lambda c=c, qt=qt, nt=nt: wo_qt(c, qt, nt))
                            )
                    return units

                # chunk 0 projected serially up front, then each attention(c)
                # drip-feeds proj(c+1) + wo(c-1) into its PE stream.  x chunks
                # are prefetched one full iteration ahead.
                xcs = {0: xc0}
                xc1 = xw.tile([P, NKT, CW], BF16, tag="xc")
                xcs[1] = xc1
                for ucost, u in proj_units(xc0, 0):
                    u()
                    sched["pe"] += ucost
                nc.sync.dma_start(out=cost[:, CW:S], in_=cos_d.ap()[:, CW:S])
                nc.scalar.dma_start(out=sint[:, CW:S], in_=sin_d.ap()[:, CW:S])
                nc.sync.dma_start(out=xc1[:], in_=xT_v[:, :, CW : 2 * CW])
                nc.scalar.dma_start(out=wo_sb[:], in_=wo_v[:])
                for c in range(NSC):
                    units = []
                    if c + 1 < NSC:
                        units += proj_units(xcs[c + 1], c + 1)
                    if c > 0:
                        normalize(c - 1)
                    # wo lags attention by 2 blocks so the last (exp-heaviest)
                    # attention block has enough PE filler
                    if c == 3:
                        units += wo_units(0) + wo_units(1) + wo_units(2)
                    if c + 2 < NSC:
                        xcn = xw.tile([P, NKT, CW], BF16, tag="xc")
                        xcs[c + 2] = xcn
                        units.append(
                            (0, lambda xcn=xcn, c=c: nc.sync.dma_start(
                                out=xcn[:],
                                in_=xT_v[:, :, (c + 2) * CW : (c + 3) * CW],
                            ))
                        )
                    if c == NSC - 1:
                        attention(
                            c,
                            units,
                            post_head=lambda dtb, hh: normalize_head_fast(
                                c, dtb, hh
                            ),
                        )
                    else:
                        attention(c, units)
                for ucost, u in wo_units(NSC - 1):
                    u()
                if DBG:
                    nc.sync.dma_start(out=dbg_ktb.ap(), in_=KTb[:])
                    nc.sync.dma_start(out=dbg_qtb.ap(), in_=QTb[:])
                    nc.sync.dma_start(out=dbg_v.ap(), in_=V[:])
                    nc.sync.dma_start(out=dbg_attnT.ap(), in_=attnT[:])
                    nc.sync.dma_start(out=dbg_ltile.ap(), in_=ltile[:])

    nc.compile()
    return nc


def _perm_a():
    """Column permutation for wq/wk: even head-dims of all heads first
    (head-major, 32 per head), then odd head-dims."""
    perm = np.empty(DG, dtype=np.int64)
    for n in range(DG):
        if n < DG // 2:
            h, i = n // 32, n % 32
            perm[n] = h * HD + 2 * i
        else:
            h, i = (n - DG // 2) // 32, (n - DG // 2) % 32
            perm[n] = h * HD + 2 * i + 1
    return perm


def _pack_w(w):
    """[D, DG] -> [NDT, P(contraction row), NKT, P(col)] so per-dt loads have
    2KB-contiguous runs on both DMA sides."""
    w4 = w.reshape(NKT, P, NDT, P)           # (kt, p, dt, m)
    return np.ascontiguousarray(w4.transpose(2, 1, 0, 3))


def kernel(**inputs):
    global _PROGRAM
    x = np.asarray(inputs["x"], dtype=np.float32)
    freqs_cos = np.asarray(inputs["freqs_cos"], dtype=np.float32)
    freqs_sin = np.asarray(inputs["freqs_sin"], dtype=np.float32)
    wq = np.asarray(inputs["wq"], dtype=np.float32)
    wk = np.asarray(inputs["wk"], dtype=np.float32)
    wv = np.asarray(inputs["wv"], dtype=np.float32)
    wo = np.asarray(inputs["wo"], dtype=np.float32)

    if _PROGRAM is None:
        _PROGRAM = _build_program()
    nc = _PROGRAM

    bf = ml_dtypes.bfloat16
    perm = _perm_a()
    cost = np.ascontiguousarray(np.tile(freqs_cos.T, (4, 1))).astype(bf)
    sint = np.ascontiguousarray(np.tile(freqs_sin.T, (4, 1))).astype(bf)

    in_maps = []
    for c in range(NCORES):
        b, g = c // 2, c % 2
        gsl = slice(g * DG, (g + 1) * DG)
        in_maps.append(
            {
                "xT": np.ascontiguousarray(x[b].T).astype(bf),
                "wq": _pack_w(wq[:, gsl][:, perm]).astype(bf),
                "wk": _pack_w(wk[:, gsl][:, perm]).astype(bf),
                "wv": np.ascontiguousarray(wv[:, gsl]).astype(bf),
                "wo": np.ascontiguousarray(wo[gsl, :]).astype(bf),
                "cost": cost,
                "sint": sint,
            }
        )

    res = run_bass_kernel_spmd(nc, in_maps, list(range(NCORES)))
    global _LAST_RESULTS
    _LAST_RESULTS = res.results
    y = np.empty((B, S, D), dtype=np.float32)
    for b in range(B):
        y[b] = res.results[2 * b]["y"].astype(np.float32) + res.results[
            2 * b + 1
        ]["y"].astype(np.float32)
    return y
